# revision 1
# baseline (speedup 1.0000x reference)
"""ConvNet+Izhikevich spiking net on 8 TRN2 NeuronCores.

Data-parallel over batch: B=16 -> 8 cores x B_shard=2. Per core the full
activation sequence [T=32, Bs=2, 256, 256] lives in SBUF (in-place across
layers). 3x3 conv is done as banded matmuls on the TensorEngine (contraction
over H in 128-row chunks, W-shifts via padded free-dim offsets); the
Izhikevich scan runs on DVE+ACT with a 6-DVE-op/2-ACT-op step:

  s  = Square(v + 112.5)                 (ACT)
  b  = -0.25*ub + j                      (DVE STT; j = 0.25*conv, in PSUM)
  t  = 0.995*ub + 1.83125                (ACT copy)
  ub = 0.001*v + t                       (DVE STT)
  v  = 0.01*s + b                        (DVE STT)
  z  = v > 30                            (DVE TS, written into act buffer)
  v  = z ? -65 : v                       (DVE copy_predicated)
  ub = 6*z + ub                          (DVE STT)

where ub = u + 366.25 (state offset folds all remaining constants).
"""

import numpy as np

T, BS, H, W, P = 32, 2, 256, 256, 128
NCH = H // P          # 2 partition chunks of H
FR = W + 2            # padded frame row: 258
FREE = NCH * BS * FR  # 1032 free elems per act frame
NL = 4
KSC = 0.25            # TAU_INV * DT

_CACHE = {}


def _build(use_f32r=True):
    import concourse.bass as bass
    import concourse.bacc as bacc
    import concourse.mybir as mybir
    from concourse.tile import TileContext

    f32 = mybir.dt.float32
    Alu = mybir.AluOpType
    Act = mybir.ActivationFunctionType

    nc = bacc.Bacc("TRN2", target_bir_lowering=False)
    x_d = nc.dram_tensor("x", [T, BS, H, W], f32, kind="ExternalInput")
    bands_d = nc.dram_tensor("bands", [NL, 3, 3, P, P], f32, kind="ExternalInput")
    out_d = nc.dram_tensor("out", [T, BS, H, W], f32, kind="ExternalOutput")

    x_r = x_d.ap().rearrange("t b (c p) w -> t b p c w", p=P)
    o_r = out_d.ap().rearrange("t b (c p) w -> t b p c w", p=P)
    bands_r = bands_d.ap().rearrange("l dw blk k m -> k l dw blk m")

    f32r = mybir.dt.float32r if use_f32r else f32

    with TileContext(nc) as tc:
        with (
            tc.tile_pool(name="acts", bufs=T) as act_pool,
            tc.tile_pool(name="state", bufs=1) as st_pool,
            tc.tile_pool(name="tmp", bufs=1) as tmp_pool,
            tc.tile_pool(name="consts", bufs=1) as c_pool,
            tc.tile_pool(name="psum", bufs=4, space="PSUM") as ps_pool,
        ):
            acts = [
                act_pool.tile([P, FREE], f32r, tag="acts", name=f"act{i}")
                for i in range(T)
            ]
            v = st_pool.tile([P, NCH * BS * W], f32, tag="v")
            u = st_pool.tile([P, NCH * BS * W], f32, tag="u")
            s = tmp_pool.tile([P, NCH * BS * W], f32, tag="s")
            b2 = tmp_pool.tile([P, NCH * BS * W], f32, tag="b2")
            t2 = tmp_pool.tile([P, NCH * BS * W], f32, tag="t2")
            ctile = c_pool.tile([P, NCH * BS * W], f32, tag="ctile")
            bands = c_pool.tile([P, NL * 3 * 3 * P], f32r, tag="bands")
            bands_st = c_pool.tile([P, NL * 3 * 3 * P], f32, tag="bands_st")
            sqbias = c_pool.tile([P, 1], f32, tag="sqbias")

            nc.vector.memset(ctile[:, :], -65.0)
            nc.vector.memset(sqbias[:, :], 112.5)
            nc.gpsimd.dma_start(
                bands_st.rearrange("p (l dw blk m) -> p l dw blk m", l=NL, dw=3, blk=3),
                bands_r,
            )
            nc.scalar.activation(bands[:, :], bands_st[:, :], Act.Copy)

            # stage x frames in f32 (zeroed pads), cast-copy into f32r act tiles
            # frame free layout: (b, c, wpad): b*2*FR + c*FR + 1 + w
            sts = [
                act_pool.tile([P, FREE], f32, tag="stage", bufs=2, name=f"st{i}")
                for i in range(2)
            ]
            nc.vector.memset(sts[0][:, :], 0.0)
            nc.vector.memset(sts[1][:, :], 0.0)
            for t in range(T):
                st = sts[t % 2]
                for b in range(BS):
                    dst = st.rearrange("p (c b wp) -> p c b wp", c=NCH, b=BS)[
                        :, :, b, 1 : 1 + W
                    ]
                    nc.gpsimd.dma_start(dst, x_r[t, b])
                nc.scalar.activation(acts[t][:, :], st[:, :], Act.Copy)

            def interior(t):
                return acts[t].rearrange("p (g wp) -> p g wp", wp=FR)[
                    :, :, 1 : 1 + W
                ]

            v3 = v.rearrange("p (g w) -> p g w", w=W)
            u3 = u.rearrange("p (g w) -> p g w", w=W)
            c3 = ctile.rearrange("p (g w) -> p g w", w=W)

            for l in range(NL):
                tc.strict_bb_all_engine_barrier()
                nc.vector.memset(v[:, :], -70.0)
                nc.vector.memset(u[:, :], 352.25)
                for t in range(T):
                    ps = ps_pool.tile([P, NCH * BS * W], f32, tag="ps")
                    for mc in range(NCH):
                        idx = 0
                        for kc in range(NCH):
                            blk = 0 if kc == mc else (1 if kc == mc - 1 else 2)
                            for dw in range(3):
                                lo = ((l * 3 + dw) * 3 + blk) * P
                                lhsT = bands[:, lo : lo + P]
                                rhs = acts[t].rearrange(
                                    "p (c bwp) -> p c bwp", c=NCH
                                )[:, kc, :].rearrange(
                                    "p (b wp) -> p b wp", b=BS
                                )[:, :, dw : dw + W]
                                nc.tensor.matmul(
                                    ps[:, mc * BS * W : (mc + 1) * BS * W],
                                    lhsT,
                                    rhs,
                                    start=(idx == 0),
                                    stop=(idx == 5),
                                )
                                idx += 1
                    zfull = interior(t)
                    HF = NCH * BS * W // 2
                    for h in range(2):
                        sl = slice(h * HF, (h + 1) * HF)
                        zr = zfull[:, h * BS : (h + 1) * BS, :]
                        nc.scalar.activation(
                            s[:, sl], v[:, sl], Act.Square, bias=sqbias[:, 0:1]
                        )
                        nc.vector.scalar_tensor_tensor(
                            b2[:, sl], u[:, sl], -0.25, ps[:, sl], Alu.mult, Alu.add
                        )
                        nc.scalar.activation(
                            t2[:, sl], u[:, sl], Act.Copy, bias=1.83125, scale=0.995
                        )
                        nc.vector.scalar_tensor_tensor(
                            u[:, sl], v[:, sl], 0.001, t2[:, sl], Alu.mult, Alu.add
                        )
                        nc.vector.scalar_tensor_tensor(
                            v[:, sl], s[:, sl], 0.01, b2[:, sl], Alu.mult, Alu.add
                        )
                        nc.vector.tensor_scalar(zr, v3[:, h * BS : (h + 1) * BS, :], 30.0, None, Alu.is_gt)
                        nc.vector.copy_predicated(
                            v3[:, h * BS : (h + 1) * BS, :],
                            zr.bitcast(mybir.dt.int32),
                            c3[:, h * BS : (h + 1) * BS, :],
                        )
                        nc.vector.scalar_tensor_tensor(
                            u3[:, h * BS : (h + 1) * BS, :], zr, 6.0,
                            u3[:, h * BS : (h + 1) * BS, :], Alu.mult, Alu.add
                        )
            for t in range(T):
                for b in range(BS):
                    src = acts[t].rearrange("p (c b wp) -> p c b wp", c=NCH, b=BS)[
                        :, :, b, 1 : 1 + W
                    ]
                    nc.sync.dma_start(o_r[t, b], src.bitcast(f32))
    if not nc.is_finalized():
        nc.finalize()
    return nc


def make_bands(w):
    """w: [4,1,1,3,3] -> [4,3,3,128,128] banded lhsT blocks, prescaled by k."""
    bands = np.zeros((NL, 3, 3, P, P), np.float32)
    for l in range(NL):
        k3 = w[l, 0, 0] * KSC
        for dw in range(3):
            D = np.zeros((P, P), np.float32)
            for dh in range(3):
                D += k3[dh, dw] * np.eye(P, k=-(dh - 1), dtype=np.float32)
            bands[l, dw, 0] = D
            bands[l, dw, 1][P - 1, 0] = k3[0, dw]  # kc = mc-1 cross block
            bands[l, dw, 2][0, P - 1] = k3[2, dw]  # kc = mc+1 cross block
    return bands


def kernel(x, weights):
    from concourse.bass_utils import run_bass_kernel_spmd

    key = "nc"
    if key not in _CACHE:
        _CACHE[key] = _build()
    nc = _CACHE[key]

    bands = make_bands(np.asarray(weights))
    xs = np.asarray(x)  # [32,16,1,256,256]
    n_cores = 8
    in_maps = []
    for c in range(n_cores):
        shard = np.ascontiguousarray(xs[:, c * BS : (c + 1) * BS, 0])
        in_maps.append({"x": shard, "bands": bands})
    res = run_bass_kernel_spmd(nc, in_maps, core_ids=list(range(n_cores)))
    out = np.zeros((T, 16, 1, H, W), np.float32)
    for c in range(n_cores):
        out[:, c * BS : (c + 1) * BS, 0] = res.results[c]["out"]
    return out



# revision 3
# speedup vs baseline: 1.0535x; 1.0535x over previous
"""ConvNet+Izhikevich v2: fused custom-DVE step, shifted state.

Shifted state: W = v + 65 (reset -> 0), G = -0.25u - 4. Then
  ps  = 0.25*conv(z) + G          (PE: 12 banded matmuls + identity matmul)
  W'  = 0.01*W^2 + 0.95*W + ps    (custom DVE op, fused threshold+reset)
  z   = (W' == 0)                 (DVE tensor_scalar, 2x mode)
  q   = (-0.00025/0.995)*W + G    (Pool STT)
  r   = 0.995*q - 0.00375         (ACT affine copy)
  G'  = -1.5*z + r                (DVE STT, 2x mode)
"""

import numpy as np

T, BS, H, W, P = 32, 2, 256, 256, 128
NCH = H // P          # 2 partition chunks of H
FR = W + 2            # padded frame row: 258
FREE = NCH * BS * FR  # 1032 free elems per act frame
NFREE = NCH * BS * W  # 1024 interior elems
NL = 4
KSC = 0.25            # TAU_INV * DT
NBLK = NL * 9 + 1     # 36 conv lhsT blocks + identity

_CACHE = {}


def _register_op(name, spec):
    from concourse import dve_ops
    from concourse.dve_ops import DveOp
    from concourse.dve_spec import lower, _has_src1
    from concourse.dve_uop import DveOpSpec

    for op in dve_ops.OPS:
        if op.name == name:
            return op
    shas = {}
    for ver in ("v3", "v4"):
        sp = DveOpSpec(name=name, opcode=0, uops=lower(spec, ver=ver),
                       rd1_en=_has_src1(spec))
        shas[ver] = sp.sha(ver)
    op = DveOp(name, spec, False, shas)
    dve_ops.OPS.append(op)
    dve_ops.CUSTOM_DVE_SPECS[name] = spec
    dve_ops._SUB_OPCODE_FOR_NAME[name] = max(dve_ops._SUB_OPCODE_FOR_NAME.values()) + 1
    return op


def _izh_ops():
    from concourse.dve_spec import Spec, Src0, Src1, C0, C1, C2, Zero, select, sq, eq

    w = sq(Src0) * C0 + Src0 * C1 + Src1
    body_w = select(w > C2, Zero, w)

    def ref_w(in0, in1, s0, s1, imm2):
        wv = (in0.astype(np.float32) * in0 * s0 + in0 * s1 + in1).astype(np.float32)
        return np.where(wv > imm2, np.float32(0.0), wv).astype(np.float32)

    a = Src1 * C1 + C2
    body_g = select(eq(Src0, Zero), a + C0, a)

    def ref_g(in0, in1, s0, s1, imm2):
        av = (in1.astype(np.float32) * s1 + imm2).astype(np.float32)
        return np.where(in0 == 0.0, av + s0, av).astype(np.float32)

    op_w = _register_op("IZH_W_ANT", Spec(body=body_w, reference=ref_w))
    op_g = _register_op("IZH_G_ANT", Spec(body=body_g, reference=ref_g))
    return op_w, op_g


def _build():
    import concourse.bass as bass
    import concourse.bacc as bacc
    import concourse.mybir as mybir
    from concourse.tile import TileContext

    izh_w, izh_g = _izh_ops()

    f32 = mybir.dt.float32
    f32r = mybir.dt.float32r
    Alu = mybir.AluOpType
    Act = mybir.ActivationFunctionType

    nc = bacc.Bacc("TRN2", target_bir_lowering=False)
    x_d = nc.dram_tensor("x", [T, BS, H, W], f32, kind="ExternalInput")
    bands_d = nc.dram_tensor("bands", [NBLK, P, P], f32, kind="ExternalInput")
    out_d = nc.dram_tensor("out", [T, BS, H, W], f32, kind="ExternalOutput")

    x_r = x_d.ap().rearrange("t b (c p) w -> t b p c w", p=P)
    o_r = out_d.ap().rearrange("t b (c p) w -> t b p c w", p=P)
    bands_r = bands_d.ap().rearrange("i k m -> k i m")

    CWQ = -0.00025 / 0.995

    with TileContext(nc) as tc:
        with (
            tc.tile_pool(name="acts", bufs=T) as act_pool,
            tc.tile_pool(name="state", bufs=1) as st_pool,
            tc.tile_pool(name="tmp", bufs=2) as tmp_pool,
            tc.tile_pool(name="consts", bufs=1) as c_pool,
            tc.tile_pool(name="psum", bufs=4, space="PSUM") as ps_pool,
        ):
            acts = [
                act_pool.tile([P, FREE], f32r, tag="acts", name=f"act{i}")
                for i in range(T)
            ]
            Wt = [st_pool.tile([P, NFREE], f32, tag=f"W{i}", name=f"W{i}") for i in range(2)]
            Gt = [st_pool.tile([P, NFREE], f32r, tag=f"G{i}", name=f"G{i}") for i in range(2)]
            qs = [tmp_pool.tile([P, NFREE], f32, tag="q", bufs=2, name=f"q{i}") for i in range(2)]
            a1s = [tmp_pool.tile([P, NFREE], f32, tag="a1", bufs=2, name=f"a1{i}") for i in range(2)]
            bands = c_pool.tile([P, NBLK * P], f32r, tag="bands")
            bands_st = c_pool.tile([P, NBLK * P], f32, tag="bands_st")

            nc.sync.dma_start(
                bands_st.rearrange("p (i m) -> p i m", i=NBLK), bands_r
            )
            nc.scalar.activation(bands[:, :], bands_st[:, :], Act.Copy)

            # stage x frames in f32 (zeroed pads), cast-copy into f32r acts
            sts = [
                act_pool.tile([P, FREE], f32, tag="stage", bufs=2, name=f"st{i}")
                for i in range(2)
            ]
            nc.gpsimd.memset(sts[0][:, :], 0.0)
            nc.gpsimd.memset(sts[1][:, :], 0.0)
            for t in range(T):
                st = sts[t % 2]
                for b in range(BS):
                    dst = st.rearrange("p (c b wp) -> p c b wp", c=NCH, b=BS)[
                        :, :, b, 1 : 1 + W
                    ]
                    nc.sync.dma_start(dst, x_r[t, b])
                nc.scalar.activation(acts[t][:, :], st[:, :], Act.Copy)

            def interior(t):
                return acts[t].rearrange("p (g wp) -> p g wp", wp=FR)[:, :, 1 : 1 + W]

            for l in range(NL):
                nc.gpsimd.memset(Wt[0][:, :], -5.0)
                nc.gpsimd.memset(Gt[0].bitcast(f32)[:, :], -0.5)
                for t in range(T):
                    Wo, Wn = Wt[t % 2], Wt[(t + 1) % 2]
                    Go, Gn = Gt[t % 2], Gt[(t + 1) % 2]
                    q, a1 = qs[t % 2], a1s[t % 2]
                    ps = ps_pool.tile([P, NFREE], f32, tag="ps")
                    for mc in range(NCH):
                        idx = 0
                        for kc in range(NCH):
                            blk = 0 if kc == mc else (1 if kc == mc - 1 else 2)
                            for dw in range(3):
                                lo = ((l * 3 + dw) * 3 + blk) * P
                                lhsT = bands[:, lo : lo + P]
                                rhs = acts[t].rearrange(
                                    "p (c bwp) -> p c bwp", c=NCH
                                )[:, kc, :].rearrange(
                                    "p (b wp) -> p b wp", b=BS
                                )[:, :, dw : dw + W]
                                nc.tensor.matmul(
                                    ps[:, mc * BS * W : (mc + 1) * BS * W],
                                    lhsT,
                                    rhs,
                                    start=(idx == 0),
                                    stop=False,
                                )
                                idx += 1
                    # += G (identity lhsT block), ends each bank's group
                    for mc in range(NCH):
                        sl = slice(mc * BS * W, (mc + 1) * BS * W)
                        nc.tensor.matmul(
                            ps[:, sl],
                            bands[:, NL * 9 * P : (NL * 9 + 1) * P],
                            Go[:, sl],
                            start=False,
                            stop=True,
                        )
                    # u-path: q = (-0.00025/0.995)*W + G  (Pool)
                    nc.scalar.activation(a1[:, :], Wo[:, :], Act.Copy, scale=CWQ)
                    nc.gpsimd.tensor_tensor(
                        q[:, :], a1[:, :], Go.bitcast(f32)[:, :], Alu.add
                    )
                    # fused v-step: W' = select(0.01W^2+0.95W+ps > 95, 0, .)
                    nc.vector._custom_dve(
                        izh_w, out=Wn[:, :], in0=Wo[:, :], in1=ps[:, :],
                        s0=0.01, s1=0.95, imm2=95.0,
                    )
                    # G' = 0.995q - 0.00375 - 1.5*(W''==0), one fused op
                    nc.vector._custom_dve(
                        izh_g, out=Gn[:, :], in0=Wn[:, :], in1=q[:, :],
                        s0=-1.5, s1=0.995, imm2=-0.00375,
                    )
                    # spike frame for next layer (off critical chain);
                    # halves split across DVE and Pool to balance load
                    zr = interior(t)
                    nc.vector.tensor_scalar(zr, Wn[:, :], 0.0, None, Alu.is_equal)
                    if l == NL - 1:
                        for b in range(BS):
                            src = acts[t].rearrange(
                                "p (c b wp) -> p c b wp", c=NCH, b=BS
                            )[:, :, b, 1 : 1 + W]
                            nc.sync.dma_start(o_r[t, b], src.bitcast(f32))
    if not nc.is_finalized():
        nc.finalize()
    return nc


def make_bands(w):
    """w: [4,1,1,3,3] -> [37,128,128]: banded lhsT blocks scaled by k, + I."""
    bands = np.zeros((NBLK, P, P), np.float32)
    for l in range(NL):
        k3 = w[l, 0, 0] * KSC
        for dw in range(3):
            D = np.zeros((P, P), np.float32)
            for dh in range(3):
                D += k3[dh, dw] * np.eye(P, k=-(dh - 1), dtype=np.float32)
            bands[(l * 3 + dw) * 3 + 0] = D
            bands[(l * 3 + dw) * 3 + 1][P - 1, 0] = k3[0, dw]  # kc = mc-1
            bands[(l * 3 + dw) * 3 + 2][0, P - 1] = k3[2, dw]  # kc = mc+1
    bands[NL * 9] = np.eye(P, dtype=np.float32)
    return bands


def kernel(x, weights):
    from concourse.bass_utils import run_bass_kernel_spmd

    key = "nc"
    if key not in _CACHE:
        _CACHE[key] = _build()
    nc = _CACHE[key]

    bands = make_bands(np.asarray(weights))
    xs = np.asarray(x)  # [32,16,1,256,256]
    n_cores = 8
    in_maps = []
    for c in range(n_cores):
        shard = np.ascontiguousarray(xs[:, c * BS : (c + 1) * BS, 0])
        in_maps.append({"x": shard, "bands": bands})
    res = run_bass_kernel_spmd(nc, in_maps, core_ids=list(range(n_cores)))
    out = np.zeros((T, 16, 1, H, W), np.float32)
    for c in range(n_cores):
        out[:, c * BS : (c + 1) * BS, 0] = res.results[c]["out"]
    return out
